# revision 6
# baseline (speedup 1.0000x reference)
"""Trainium2 Bass kernel for gnn_message_passing (nn_CGTPEL_72645076844777).

Strategy (edge-parallel over 8 cores, per the sharding hint):
 - Host: sort edges by src; core i owns edges whose src is in node range
   [i*1250, (i+1)*1250). Gather node_attr[dst] per shard, pad shards to a
   common size, bake a uniform sliding-window schedule so one SPMD program
   serves all cores.
 - Device (per core): per 128-edge tile, PE computes the two FC matmuls
   (W2 resident in SBUF, per-tile hT as stationary); the e3nn tensor
   product is a per-edge bilinear contraction done on the vector engine
   with broadcast access patterns; the b2-bias contribution to the TP is
   folded into a dense matmul against a host-built 256x128 matrix.
   Scatter-sum over edge_src is a one-hot matmul accumulated in PSUM over
   a sliding 512-node window (edges sorted by src make windows contiguous).
   BatchNorm statistics are summed with a ones-matmul; a 96-float
   AllReduce provides global stats; each core normalizes and writes its
   1250-node slice of the output.
"""
import numpy as np

MUL = 32
P = 128
EPS = 1e-5
INV_SQRT3 = 1.0 / np.sqrt(3.0)
PATH_NORM = 1.0 / np.sqrt(2.0 * MUL)
N_CORES = 8
WIN = 512
CHK = 512

_CACHE = {}


# ----------------------------------------------------------------- host prep
def host_prep(inputs, win=WIN, chk=CHK, n_cores=N_CORES):
    node_attr = np.ascontiguousarray(np.asarray(inputs["node_attr"], np.float32))
    edge_index = np.asarray(inputs["edge_index"]).astype(np.int64)
    edge_attr = np.asarray(inputs["edge_attr"], np.float32)
    edge_sh = np.asarray(inputs["edge_sh"], np.float32)
    W1 = np.asarray(inputs["W1"], np.float32)
    b1 = np.asarray(inputs["b1"], np.float32)
    W2 = np.asarray(inputs["W2"], np.float32)
    b2 = np.asarray(inputs["b2"], np.float32)
    bnw = np.asarray(inputs["bn_weight"], np.float32)
    bnb = np.asarray(inputs["bn_bias"], np.float32)

    N = node_attr.shape[0]
    assert N % n_cores == 0
    n_c = N // n_cores

    src, dst = edge_index[0], edge_index[1]
    order = np.argsort(src, kind="stable")
    src_s, dst_s = src[order], dst[order]

    starts = np.searchsorted(src_s, np.arange(0, N + 1, n_c))
    e_counts = np.diff(starts)
    E_pad = int(np.ceil(max(e_counts.max(), 1) / P) * P)
    T = E_pad // P

    # per-core local src, padded (pads point at last local node, contribute 0)
    locs = np.full((n_cores, E_pad), n_c - 1, np.int64)
    for ci in range(n_cores):
        sl = slice(starts[ci], starts[ci + 1])
        locs[ci, :e_counts[ci]] = src_s[sl] - ci * n_c

    # uniform window schedule covering every core's tile ranges
    tl = locs.reshape(n_cores, T, P)
    lo_t = tl.min(axis=(0, 2))
    hi_t = tl.max(axis=(0, 2))
    assert (hi_t - lo_t < win).all(), "window too small for tile spread"
    wb = np.clip((lo_t + hi_t + 1) // 2 - win // 2, 0, n_c - win).astype(np.int64)
    wb = np.maximum.accumulate(wb)  # monotone
    assert (lo_t >= wb).all() and (hi_t < wb + win).all()

    n_chunks = int(np.ceil(n_c / chk))
    first_t = np.full(n_chunks, T, np.int64)
    last_t = np.full(n_chunks, -1, np.int64)
    for t in range(T):
        for c in range(n_chunks):
            lo, hi = c * chk, min((c + 1) * chk, n_c)
            if wb[t] < hi and wb[t] + win > lo:
                first_t[c] = min(first_t[c], t)
                last_t[c] = max(last_t[c], t)
    assert first_t[0] == 0 and last_t[-1] == T - 1
    for c in range(2, n_chunks):
        assert first_t[c] > last_t[c - 2], "psum chunk ring-2 violated"

    # fold path normalization into W2 / b2
    scale = np.full(4, PATH_NORM * INV_SQRT3, np.float32)
    scale[0] = PATH_NORM
    W2f = (W2.reshape(128, 4, MUL * MUL) * scale[None, :, None]).reshape(128, -1)
    W2f = np.ascontiguousarray(W2f, np.float32)
    b2f = (b2.reshape(4, MUL * MUL) * scale[:, None]).reshape(4, MUL, MUL)

    b2A, b2B, b2C, b2D = b2f[0], b2f[1], b2f[2], b2f[3]
    B2comb = np.zeros((256, 128), np.float32)
    B2comb[0:32, 0:32] = b2A
    B2comb[32:64, 0:32] = b2D
    wcols = 32 + 3 * np.arange(MUL)
    for u in range(MUL):
        for i in range(3):
            B2comb[64 + 3 * u + i, wcols + i] = b2B[u]
            B2comb[160 + 3 * u + i, wcols + i] = b2C[u]
    # packed as [128, 256]: cols 0:128 = rows 0:128, cols 128:256 = rows 128:256
    B2pack = np.ascontiguousarray(
        np.concatenate([B2comb[0:128], B2comb[128:256]], axis=1), np.float32)

    iota_full = np.broadcast_to(np.arange(win, dtype=np.float32), (P, win))
    iota_full = np.ascontiguousarray(iota_full)
    cnst_row = np.zeros((1, 128), np.float32)
    cnst_row[0, 0:32] = bnw[:32]
    cnst_row[0, 32:64] = bnw[32:]
    cnst_row[0, 64:96] = bnb

    cores = []
    for ci in range(n_cores):
        sl = slice(starts[ci], starts[ci + 1])
        ec = e_counts[ci]
        ea = np.zeros((E_pad, 128), np.float32)
        xg = np.zeros((E_pad, 128), np.float32)
        shls = np.zeros((E_pad, 8), np.float32)
        ea[:ec] = edge_attr[order[sl]]
        xg[:ec] = node_attr[dst_s[sl]]
        shls[:ec, 0:4] = edge_sh[order[sl]]
        ls_adj = locs[ci] - wb[np.arange(E_pad) // P]
        assert (ls_adj >= 0).all() and (ls_adj < win).all()
        shls[:, 4] = ls_adj.astype(np.float32)
        cnt = np.bincount(locs[ci, :ec], minlength=n_c).astype(np.float32)
        inv_cnt = (1.0 / np.maximum(cnt, 1.0)).astype(np.float32)[:, None]
        resid = np.ascontiguousarray(node_attr[ci * n_c:(ci + 1) * n_c])
        cores.append({"ea": ea, "xg": xg, "shls": shls,
                      "invc": inv_cnt, "resid": resid})

    consts = {"w1": np.ascontiguousarray(W1), "b1": b1.reshape(128, 1).copy(),
              "w2": W2f, "b2p": B2pack, "iota": iota_full, "cnst": cnst_row}
    meta = dict(n_c=n_c, E_pad=E_pad, T=T, wb=tuple(int(x) for x in wb),
                n_chunks=n_chunks, first_t=tuple(int(x) for x in first_t),
                last_t=tuple(int(x) for x in last_t), N=N, win=win, chk=chk,
                n_cores=n_cores)
    return cores, consts, meta


# ------------------------------------------------------------- device program
def build_nc(meta, no_collective=False):
    import concourse.bass as bass  # noqa: F401
    import concourse.tile as tile
    from concourse import mybir, bacc
    from concourse.masks import make_identity

    f32 = mybir.dt.float32
    ALU = mybir.AluOpType
    AX = mybir.AxisListType
    AF = mybir.ActivationFunctionType

    n_c, E_pad, T = meta["n_c"], meta["E_pad"], meta["T"]
    wb, n_chunks = meta["wb"], meta["n_chunks"]
    first_t, last_t = meta["first_t"], meta["last_t"]
    win, chk, N, n_cores = meta["win"], meta["chk"], meta["N"], meta["n_cores"]

    nc = bacc.Bacc("TRN2", target_bir_lowering=False, debug=False,
                   num_devices=n_cores)

    ea_d = nc.dram_tensor("ea", [E_pad, 128], f32, kind="ExternalInput")
    xg_d = nc.dram_tensor("xg", [E_pad, 128], f32, kind="ExternalInput")
    shls_d = nc.dram_tensor("shls", [E_pad, 8], f32, kind="ExternalInput")
    w1_d = nc.dram_tensor("w1", [128, 128], f32, kind="ExternalInput")
    b1_d = nc.dram_tensor("b1", [128, 1], f32, kind="ExternalInput")
    w2_d = nc.dram_tensor("w2", [128, 4096], f32, kind="ExternalInput")
    b2p_d = nc.dram_tensor("b2p", [128, 256], f32, kind="ExternalInput")
    iota_d = nc.dram_tensor("iota", [P, win], f32, kind="ExternalInput")
    cnst_d = nc.dram_tensor("cnst", [1, 128], f32, kind="ExternalInput")
    invc_d = nc.dram_tensor("invc", [n_c, 1], f32, kind="ExternalInput")
    resid_d = nc.dram_tensor("resid", [n_c, 128], f32, kind="ExternalInput")
    out_d = nc.dram_tensor("out", [n_c, 128], f32, kind="ExternalOutput")

    n_node_tiles = (n_c + P - 1) // P

    with tile.TileContext(nc, num_cores=n_cores) as tc:
        with (
            tc.tile_pool(name="const", bufs=1) as cst,
            tc.tile_pool(name="io", bufs=3) as io,
            tc.tile_pool(name="sb", bufs=2) as sb,
            tc.tile_pool(name="xbp", bufs=n_node_tiles) as xbp,
            tc.tile_pool(name="pss", bufs=2, space="PSUM") as pss,
            tc.tile_pool(name="psw", bufs=2, space="PSUM") as psw,
            tc.tile_pool(name="pscat", bufs=2, space="PSUM") as pscat,
            tc.tile_pool(name="dram", bufs=1, space="DRAM") as dram,
        ):
            # ---- constants
            w1_sb = cst.tile([128, 128], f32, tag="w1")
            nc.sync.dma_start(out=w1_sb[:], in_=w1_d[:])
            b1_sb = cst.tile([128, 1], f32, tag="b1")
            nc.sync.dma_start(out=b1_sb[:], in_=b1_d[:])
            w2_sb = cst.tile([128, 4096], f32, tag="w2")
            nc.sync.dma_start(out=w2_sb[:], in_=w2_d[:])
            b2p_sb = cst.tile([128, 256], f32, tag="b2p")
            nc.sync.dma_start(out=b2p_sb[:], in_=b2p_d[:])
            iota_sb = cst.tile([P, win], f32, tag="iota")
            nc.sync.dma_start(out=iota_sb[:], in_=iota_d[:])
            cnst_sb = cst.tile([1, 128], f32, tag="cnst")
            nc.sync.dma_start(out=cnst_sb[:], in_=cnst_d[:])
            ident = cst.tile([128, 128], f32, tag="ident")
            make_identity(nc, ident[:])
            zeros_sb = cst.tile([128, chk], f32, tag="zeros")
            nc.gpsimd.memset(zeros_sb[:], 0.0)
            ones_sb = cst.tile([128, 1], f32, tag="ones")
            nc.gpsimd.memset(ones_sb[:], 1.0)
            stats_acc = cst.tile([96, 1], f32, tag="stacc")
            nc.gpsimd.memset(stats_acc[:], 0.0)

            chunk_tiles = [None] * n_chunks
            xb_tiles = []
            xb_rows = []

            def finalize_chunk(c):
                nvalid = min(chk, n_c - c * chk)
                cs = sb.tile([128, chk], f32, tag="chfin")
                nc.scalar.copy(cs[:, 0:nvalid], chunk_tiles[c][:, 0:nvalid])
                nsub = (nvalid + P - 1) // P
                for j in range(nsub):
                    rows = min(P, nvalid - j * P)
                    node0 = c * chk + j * P
                    ntp = pss.tile([128, 128], f32, tag="pss")
                    nc.tensor.transpose(
                        out=ntp[0:rows, :], in_=cs[:, j * P:j * P + rows],
                        identity=ident[:])
                    invc_t = io.tile([128, 1], f32, tag="invc")
                    nc.sync.dma_start(out=invc_t[0:rows, :],
                                      in_=invc_d[node0:node0 + rows, :])
                    resid_t = io.tile([128, 128], f32, tag="resid")
                    nc.sync.dma_start(out=resid_t[0:rows, :],
                                      in_=resid_d[node0:node0 + rows, :])
                    xb = xbp.tile([128, 128], f32, tag="xb")
                    nc.vector.scalar_tensor_tensor(
                        out=xb[0:rows, :], in0=ntp[0:rows, :],
                        scalar=invc_t[0:rows, 0:1], in1=resid_t[0:rows, :],
                        op0=ALU.mult, op1=ALU.add)
                    xb_tiles.append(xb)
                    xb_rows.append((node0, rows))
                    # stats block [rows, 96] = [s | s^2 | sum_i v^2]
                    stt = sb.tile([128, 96], f32, tag="stt")
                    nc.vector.tensor_copy(stt[0:rows, 0:32], xb[0:rows, 0:32])
                    nc.vector.tensor_tensor(
                        out=stt[0:rows, 32:64], in0=xb[0:rows, 0:32],
                        in1=xb[0:rows, 0:32], op=ALU.mult)
                    v2 = sb.tile([128, 96], f32, tag="v2")
                    nc.vector.tensor_tensor(
                        out=v2[0:rows, :], in0=xb[0:rows, 32:128],
                        in1=xb[0:rows, 32:128], op=ALU.mult)
                    nc.vector.tensor_reduce(
                        out=stt[0:rows, 64:96],
                        in_=v2[0:rows, :].rearrange("e (u i) -> e u i", u=32, i=3),
                        axis=AX.X, op=ALU.add)
                    stp = pss.tile([96, 1], f32, tag="pss")
                    nc.tensor.matmul(
                        out=stp[:], lhsT=stt[0:rows, 0:96],
                        rhs=ones_sb[0:rows, 0:1], start=True, stop=True)
                    nc.vector.tensor_tensor(
                        out=stats_acc[:], in0=stats_acc[:], in1=stp[:],
                        op=ALU.add)

            # ---------------- main edge-tile loop
            for t in range(T):
                ea_t = io.tile([128, 128], f32, tag="ea")
                nc.sync.dma_start(out=ea_t[:], in_=ea_d[t * P:(t + 1) * P, :])
                xg_t = io.tile([128, 128], f32, tag="xg")
                nc.sync.dma_start(out=xg_t[:], in_=xg_d[t * P:(t + 1) * P, :])
                shls_t = io.tile([128, 8], f32, tag="shls")
                nc.sync.dma_start(out=shls_t[:], in_=shls_d[t * P:(t + 1) * P, :])

                # PE: eaT -> mm1 -> relu
                eaT_ps = pss.tile([128, 128], f32, tag="pss")
                nc.tensor.transpose(out=eaT_ps[:], in_=ea_t[:], identity=ident[:])
                eaT_sb = sb.tile([128, 128], f32, tag="eaT")
                nc.scalar.copy(eaT_sb[:], eaT_ps[:])
                hT_ps = pss.tile([128, 128], f32, tag="pss")
                nc.tensor.matmul(out=hT_ps[:], lhsT=w1_sb[:], rhs=eaT_sb[:],
                                 start=True, stop=True)
                hT_sb = sb.tile([128, 128], f32, tag="hT")
                nc.scalar.activation(hT_sb[:], hT_ps[:], AF.Relu,
                                     bias=b1_sb[:, 0:1])

                # DVE: V prep
                V = sb.tile([128, 256], f32, tag="V")
                x0 = xg_t[:, 0:32]
                x1v = xg_t[:, 32:128].rearrange("e (u i) -> e u i", u=32, i=3)
                sh0 = shls_t[:, 0:1]
                sh1u = shls_t[:, 1:4].unsqueeze(1).broadcast_to([P, 32, 3])
                y2t = sb.tile([128, 96], f32, tag="y2t")
                y2tv = y2t[:].rearrange("e (u i) -> e u i", u=32, i=3)
                nc.vector.tensor_tensor(out=y2tv, in0=x1v, in1=sh1u, op=ALU.mult)
                nc.vector.tensor_reduce(out=V[:, 32:64], in_=y2tv,
                                        axis=AX.X, op=ALU.add)
                nc.vector.tensor_scalar(out=V[:, 0:32], in0=x0, scalar1=sh0,
                                        scalar2=None, op0=ALU.mult)
                x0u = x0.unsqueeze(2).broadcast_to([P, 32, 3])
                nc.vector.tensor_tensor(
                    out=V[:, 64:160].rearrange("e (u i) -> e u i", u=32, i=3),
                    in0=x0u, in1=sh1u, op=ALU.mult)
                nc.vector.tensor_scalar(out=V[:, 160:256], in0=xg_t[:, 32:128],
                                        scalar1=sh0, scalar2=None, op0=ALU.mult)

                # PE: transpose V, corr matmuls
                VT_ps = pss.tile([128, 256], f32, tag="pss")
                nc.tensor.transpose(out=VT_ps[:, 0:128], in_=V[:, 0:128],
                                    identity=ident[:])
                nc.tensor.transpose(out=VT_ps[:, 128:256], in_=V[:, 128:256],
                                    identity=ident[:])
                VT_sb = sb.tile([128, 256], f32, tag="VT")
                nc.scalar.copy(VT_sb[:], VT_ps[:])
                corr_ps = pss.tile([128, 128], f32, tag="pss")
                nc.tensor.matmul(out=corr_ps[:], lhsT=VT_sb[:, 0:128],
                                 rhs=b2p_sb[:, 0:128], start=True, stop=False)
                nc.tensor.matmul(out=corr_ps[:], lhsT=VT_sb[:, 128:256],
                                 rhs=b2p_sb[:, 128:256], start=False, stop=True)

                # mm2 + TP products per path
                red = {}
                for p in range(4):
                    wps = psw.tile([128, 1024], f32, tag="w")
                    for h in range(2):
                        nc.tensor.matmul(
                            out=wps[:, h * 512:(h + 1) * 512], lhsT=hT_sb[:],
                            rhs=w2_sb[:, p * 1024 + h * 512:p * 1024 + (h + 1) * 512],
                            start=True, stop=True)
                    if p == 2:  # path C
                        prodC = sb.tile([128, 3072], f32, tag="prod")
                        wv = wps[:].rearrange("e (u w) -> e u w", u=32, w=32)
                        wv = wv.unsqueeze(3).broadcast_to([P, 32, 32, 3])
                        vCv = V[:, 160:256].rearrange("e (u i) -> e u i", u=32, i=3)
                        vCv = vCv.unsqueeze(2).broadcast_to([P, 32, 32, 3])
                        pCv = prodC[:].rearrange(
                            "e (u w i) -> e u w i", u=32, w=32, i=3)
                        nc.vector.tensor_tensor(out=pCv, in0=wv, in1=vCv,
                                                op=ALU.mult)
                        rC = sb.tile([128, 96], f32, tag="rC")
                        nc.vector.tensor_reduce(
                            out=rC[:],
                            in_=prodC[:].rearrange(
                                "e (u w i) -> e w i u", u=32, w=32, i=3),
                            axis=AX.X, op=ALU.add)
                        red[p] = rC
                    else:
                        if p == 0:
                            vec = V[:, 0:32]
                        elif p == 1:
                            vec = xg_t[:, 0:32]
                        else:
                            vec = V[:, 32:64]
                        prod = sb.tile([128, 3072], f32, tag="prod")
                        wv = wps[:].rearrange("e (u w) -> e u w", u=32, w=32)
                        vv = vec.unsqueeze(2).broadcast_to([P, 32, 32])
                        nc.vector.tensor_tensor(
                            out=prod[:, 0:1024].rearrange(
                                "e (u w) -> e u w", u=32, w=32),
                            in0=wv, in1=vv, op=ALU.mult)
                        r = sb.tile([128, 32], f32, tag=f"r{p}")
                        nc.vector.tensor_reduce(
                            out=r[:],
                            in_=prod[:, 0:1024].rearrange(
                                "e (u w) -> e w u", u=32, w=32),
                            axis=AX.X, op=ALU.add)
                        red[p] = r

                # assembly
                scat_sb = sb.tile([128, 128], f32, tag="scat")
                o0 = sb.tile([128, 32], f32, tag="o0")
                nc.vector.tensor_tensor(out=o0[:], in0=red[0][:], in1=red[3][:],
                                        op=ALU.add)
                nc.vector.tensor_tensor(out=scat_sb[:, 0:32], in0=o0[:],
                                        in1=corr_ps[:, 0:32], op=ALU.add)
                t1 = sb.tile([128, 96], f32, tag="t1")
                cBv = red[1][:].unsqueeze(2).broadcast_to([P, 32, 3])
                nc.vector.tensor_tensor(
                    out=t1[:].rearrange("e (w i) -> e w i", w=32, i=3),
                    in0=cBv, in1=sh1u, op=ALU.mult)
                t2 = sb.tile([128, 96], f32, tag="t2")
                nc.vector.tensor_tensor(out=t2[:], in0=t1[:], in1=red[2][:],
                                        op=ALU.add)
                nc.vector.tensor_tensor(out=scat_sb[:, 32:128], in0=t2[:],
                                        in1=corr_ps[:, 32:128], op=ALU.add)

                # one-hot S and scatter matmuls
                S_sb = sb.tile([P, win], f32, tag="S")
                nc.vector.tensor_scalar(out=S_sb[:], in0=iota_sb[:],
                                        scalar1=shls_t[:, 4:5], scalar2=None,
                                        op0=ALU.is_equal)
                for c in range(n_chunks):
                    lo, hi = c * chk, min((c + 1) * chk, n_c)
                    a, b = max(wb[t], lo), min(wb[t] + win, hi)
                    if a >= b:
                        continue
                    if t == first_t[c]:
                        chunk_tiles[c] = pscat.tile([128, chk], f32, tag="ch", name=f"ch{c}")
                        nc.tensor.matmul(out=chunk_tiles[c][:],
                                         lhsT=ident[:], rhs=zeros_sb[:],
                                         start=True, stop=False)
                    nc.tensor.matmul(
                        out=chunk_tiles[c][:, a - lo:b - lo],
                        lhsT=scat_sb[:], rhs=S_sb[:, a - wb[t]:b - wb[t]],
                        start=False, stop=(t == last_t[c]))
                for c in range(n_chunks):
                    if last_t[c] == t:
                        finalize_chunk(c)

            # ---------------- tail: AllReduce of stats, normalize, write out
            arin = dram.tile([96, 1], f32, name="arin")
            arout = dram.tile([96, 1], f32, name="arout")
            nc.sync.dma_start(out=arin[:], in_=stats_acc[:])
            if no_collective:
                nc.sync.dma_start(out=arout[:], in_=arin[:])
            else:
                from concourse import mybir as _mb
                nc.gpsimd.collective_compute(
                    "AllReduce", _mb.AluOpType.add,
                    replica_groups=[list(range(n_cores))],
                    ins=[arin[:].opt()], outs=[arout[:].opt()])
            srow = sb.tile([1, 96], f32, tag="srow")
            nc.sync.dma_start(out=srow[:], in_=arout[:].rearrange("a b -> b a"))

            # constants prep on partition 0
            pr = sb.tile([1, 160], f32, tag="pr")
            mu = pr[:, 0:32]
            alpha = pr[:, 32:64]
            gamma = pr[:, 64:96]
            delta = pr[:, 96:128]
            tmp = pr[:, 128:160]
            nc.vector.tensor_scalar(out=mu, in0=srow[:, 0:32], scalar1=1.0 / N,
                                    scalar2=None, op0=ALU.mult)
            # var = S2/N - mu^2 + eps
            nc.vector.tensor_scalar(out=tmp, in0=srow[:, 32:64], scalar1=1.0 / N,
                                    scalar2=EPS, op0=ALU.mult, op1=ALU.add)
            va = sb.tile([1, 32], f32, tag="va")
            nc.vector.tensor_tensor(out=va[:], in0=mu, in1=mu, op=ALU.mult)
            nc.vector.tensor_tensor(out=tmp, in0=tmp, in1=va[:], op=ALU.subtract)
            nc.scalar.sqrt(tmp, tmp)
            nc.vector.reciprocal(tmp, tmp)
            nc.vector.tensor_tensor(out=alpha, in0=tmp, in1=cnst_sb[:, 0:32],
                                    op=ALU.mult)
            nc.vector.tensor_scalar(out=tmp, in0=srow[:, 64:96],
                                    scalar1=1.0 / (3 * N), scalar2=EPS,
                                    op0=ALU.mult, op1=ALU.add)
            nc.scalar.sqrt(tmp, tmp)
            nc.vector.reciprocal(tmp, tmp)
            nc.vector.tensor_tensor(out=gamma, in0=tmp, in1=cnst_sb[:, 32:64],
                                    op=ALU.mult)
            nc.vector.tensor_tensor(out=delta, in0=mu, in1=alpha, op=ALU.mult)
            nc.vector.tensor_tensor(out=delta, in0=delta, in1=cnst_sb[:, 64:96],
                                    op=ALU.subtract)

            rows2 = sb.tile([1, 256], f32, tag="rows2")
            nc.gpsimd.memset(rows2[:], 0.0)
            nc.vector.tensor_copy(rows2[:, 0:32], alpha)
            nc.vector.tensor_copy(
                rows2[:, 32:128].rearrange("e (u i) -> e u i", u=32, i=3),
                gamma.unsqueeze(2).broadcast_to([1, 32, 3]))
            nc.vector.tensor_copy(rows2[:, 128:160], delta)
            rowb = dram.tile([1, 256], f32, name="rowb")
            nc.sync.dma_start(out=rowb[:], in_=rows2[:])
            scaleB = cst.tile([128, 128], f32, tag="scaleB")
            nc.sync.dma_start(
                out=scaleB[:].unsqueeze(1),
                in_=rowb[0:1, 0:128].partition_broadcast(128))
            deltaB = cst.tile([128, 128], f32, tag="deltaB")
            nc.sync.dma_start(
                out=deltaB[:].unsqueeze(1),
                in_=rowb[0:1, 128:256].partition_broadcast(128))

            for xb, (node0, rows) in zip(xb_tiles, xb_rows):
                nrm = sb.tile([128, 128], f32, tag="nrm")
                nc.vector.tensor_tensor(out=nrm[0:rows, :], in0=xb[0:rows, :],
                                        in1=scaleB[0:rows, :], op=ALU.mult)
                nrm2 = sb.tile([128, 128], f32, tag="nrm2")
                nc.vector.tensor_tensor(out=nrm2[0:rows, :], in0=nrm[0:rows, :],
                                        in1=deltaB[0:rows, :], op=ALU.subtract)
                nc.sync.dma_start(out=out_d[node0:node0 + rows, :],
                                  in_=nrm2[0:rows, :])

    nc.compile()
    return nc


# ------------------------------------------------------------------ entry
_TRACE = False
_LAST = {}


def kernel(**inputs):
    from concourse.bass_utils import run_bass_kernel_spmd

    cores, consts, meta = host_prep(inputs)
    key = (meta["E_pad"], meta["wb"], meta["first_t"], meta["last_t"],
           meta["n_c"], meta["N"])
    if key not in _CACHE:
        _CACHE[key] = build_nc(meta)
    nc = _CACHE[key]

    in_maps = []
    for ci in range(meta["n_cores"]):
        m = {"ea": cores[ci]["ea"], "xg": cores[ci]["xg"],
             "shls": cores[ci]["shls"], "invc": cores[ci]["invc"],
             "resid": cores[ci]["resid"], "w1": consts["w1"],
             "b1": consts["b1"], "w2": consts["w2"], "b2p": consts["b2p"],
             "iota": consts["iota"], "cnst": consts["cnst"]}
        in_maps.append(m)
    res = run_bass_kernel_spmd(nc, in_maps,
                               core_ids=list(range(meta["n_cores"])),
                               trace=_TRACE)
    _LAST["exec_time_ns"] = res.exec_time_ns
    _LAST["profile_json"] = res.profile_json
    out = np.concatenate([res.results[ci]["out"]
                          for ci in range(meta["n_cores"])], axis=0)
    return out.astype(np.float32)


# revision 13
# speedup vs baseline: 1.2275x; 1.2275x over previous
"""Trainium2 Bass kernel for gnn_message_passing (nn_CGTPEL_72645076844777).

Strategy (edge-parallel over 8 cores, per the sharding hint):
 - Host: sort edges by src; core i owns edges whose src is in node range
   [i*1250, (i+1)*1250). Gather node_attr[dst] per shard, pad shards to a
   common size, bake a uniform sliding-window schedule so one SPMD program
   serves all cores.
 - Device (per core): per 128-edge tile, PE computes the two FC matmuls
   (W2 resident in SBUF, per-tile hT as stationary); the e3nn tensor
   product is a per-edge bilinear contraction done on the vector engine
   with broadcast access patterns; the b2-bias contribution to the TP is
   folded into a dense matmul against a host-built 256x128 matrix.
   Scatter-sum over edge_src is a one-hot matmul accumulated in PSUM over
   a sliding 512-node window (edges sorted by src make windows contiguous).
   BatchNorm statistics are summed with a ones-matmul; a 96-float
   AllReduce provides global stats; each core normalizes and writes its
   1250-node slice of the output.
"""
import numpy as np

MUL = 32
P = 128
EPS = 1e-5
INV_SQRT3 = 1.0 / np.sqrt(3.0)
PATH_NORM = 1.0 / np.sqrt(2.0 * MUL)
N_CORES = 8
WIN = 512
CHK = 512

_CACHE = {}


# ----------------------------------------------------------------- host prep
def host_prep(inputs, win=WIN, chk=CHK, n_cores=N_CORES):
    node_attr = np.ascontiguousarray(np.asarray(inputs["node_attr"], np.float32))
    edge_index = np.asarray(inputs["edge_index"]).astype(np.int64)
    edge_attr = np.asarray(inputs["edge_attr"], np.float32)
    edge_sh = np.asarray(inputs["edge_sh"], np.float32)
    W1 = np.asarray(inputs["W1"], np.float32)
    b1 = np.asarray(inputs["b1"], np.float32)
    W2 = np.asarray(inputs["W2"], np.float32)
    b2 = np.asarray(inputs["b2"], np.float32)
    bnw = np.asarray(inputs["bn_weight"], np.float32)
    bnb = np.asarray(inputs["bn_bias"], np.float32)

    N = node_attr.shape[0]
    assert N % n_cores == 0
    n_c = N // n_cores

    src, dst = edge_index[0], edge_index[1]
    order = np.argsort(src, kind="stable")
    src_s, dst_s = src[order], dst[order]

    starts = np.searchsorted(src_s, np.arange(0, N + 1, n_c))
    e_counts = np.diff(starts)
    E_pad = int(np.ceil(max(e_counts.max(), 1) / P) * P)
    T = E_pad // P

    # per-core local src, padded (pads point at last local node, contribute 0)
    locs = np.full((n_cores, E_pad), n_c - 1, np.int64)
    for ci in range(n_cores):
        sl = slice(starts[ci], starts[ci + 1])
        locs[ci, :e_counts[ci]] = src_s[sl] - ci * n_c

    # uniform window schedule covering every core's tile ranges
    tl = locs.reshape(n_cores, T, P)
    lo_t = tl.min(axis=(0, 2))
    hi_t = tl.max(axis=(0, 2))
    assert (hi_t - lo_t < win).all(), "window too small for tile spread"
    wb = np.clip((lo_t + hi_t + 1) // 2 - win // 2, 0, n_c - win).astype(np.int64)
    wb = np.maximum.accumulate(wb)  # monotone
    assert (lo_t >= wb).all() and (hi_t < wb + win).all()

    n_chunks = int(np.ceil(n_c / chk))
    first_t = np.full(n_chunks, T, np.int64)
    last_t = np.full(n_chunks, -1, np.int64)
    for t in range(T):
        for c in range(n_chunks):
            lo, hi = c * chk, min((c + 1) * chk, n_c)
            if wb[t] < hi and wb[t] + win > lo:
                first_t[c] = min(first_t[c], t)
                last_t[c] = max(last_t[c], t)
    assert first_t[0] == 0 and last_t[-1] == T - 1
    for c in range(2, n_chunks):
        assert first_t[c] > last_t[c - 2], "psum chunk ring-2 violated"

    # fold path normalization into W2 / b2
    scale = np.full(4, PATH_NORM * INV_SQRT3, np.float32)
    scale[0] = PATH_NORM
    W2f = (W2.reshape(128, 4, MUL * MUL) * scale[None, :, None]).reshape(128, -1)
    W2f = np.ascontiguousarray(W2f, np.float32)
    b2f = (b2.reshape(4, MUL * MUL) * scale[:, None]).reshape(4, MUL, MUL)

    b2A, b2B, b2C, b2D = b2f[0], b2f[1], b2f[2], b2f[3]
    B2comb = np.zeros((256, 128), np.float32)
    B2comb[0:32, 0:32] = b2A
    B2comb[32:64, 0:32] = b2D
    wcols = 32 + 3 * np.arange(MUL)
    for u in range(MUL):
        for i in range(3):
            B2comb[64 + 3 * u + i, wcols + i] = b2B[u]
            B2comb[160 + 3 * u + i, wcols + i] = b2C[u]
    # packed as [128, 256]: cols 0:128 = rows 0:128, cols 128:256 = rows 128:256
    B2pack = np.ascontiguousarray(
        np.concatenate([B2comb[0:128], B2comb[128:256]], axis=1), np.float32)

    iota_full = np.broadcast_to(np.arange(win, dtype=np.float32), (P, win))
    iota_full = np.ascontiguousarray(iota_full)
    cnst_row = np.zeros((1, 128), np.float32)
    cnst_row[0, 0:32] = bnw[:32]
    cnst_row[0, 32:64] = bnw[32:]
    cnst_row[0, 64:96] = bnb

    cores = []
    for ci in range(n_cores):
        sl = slice(starts[ci], starts[ci + 1])
        ec = e_counts[ci]
        ea = np.zeros((E_pad, 128), np.float32)
        xg = np.zeros((E_pad, 128), np.float32)
        shls = np.zeros((E_pad, 8), np.float32)
        ea[:ec] = edge_attr[order[sl]]
        xg[:ec] = node_attr[dst_s[sl]]
        shls[:ec, 0:4] = edge_sh[order[sl]]
        ls_adj = locs[ci] - wb[np.arange(E_pad) // P]
        assert (ls_adj >= 0).all() and (ls_adj < win).all()
        shls[:, 4] = ls_adj.astype(np.float32)
        cnt = np.bincount(locs[ci, :ec], minlength=n_c).astype(np.float32)
        inv_cnt = (1.0 / np.maximum(cnt, 1.0)).astype(np.float32)[:, None]
        resid = np.ascontiguousarray(node_attr[ci * n_c:(ci + 1) * n_c])
        cores.append({"ea": ea, "xg": xg, "shls": shls,
                      "invc": inv_cnt, "resid": resid})

    consts = {"w1": np.ascontiguousarray(W1), "b1": b1.reshape(128, 1).copy(),
              "w2": W2f, "b2p": B2pack, "iota": iota_full, "cnst": cnst_row}
    meta = dict(n_c=n_c, E_pad=E_pad, T=T, wb=tuple(int(x) for x in wb),
                n_chunks=n_chunks, first_t=tuple(int(x) for x in first_t),
                last_t=tuple(int(x) for x in last_t), N=N, win=win, chk=chk,
                n_cores=n_cores)
    return cores, consts, meta


# --------------------------------------------------- custom fused DVE op
def _register_mul_cumsum():
    """Register (once) a custom DVE op: out = running-sum of in0*in1 along
    the free-dim stream. Grouped sums are then strided samples + a diff."""
    import concourse.dve_ops as dve_ops
    from concourse.dve_spec import Spec, Src0, Src1, scan, AluOp, lower
    from concourse.dve_uop import DveOpSpec

    NAME = "ANT_MUL_CUMSUM"
    for op in dve_ops.OPS:
        if op.name == NAME:
            return op

    def _ref(in0, in1, c0, c1, c2):
        prod = (np.asarray(in0, np.float32) * np.asarray(in1, np.float32))
        flat = prod.reshape(prod.shape[0], -1)
        return np.cumsum(flat, axis=-1, dtype=np.float32).reshape(prod.shape)

    spec = Spec(body=scan(AluOp.ADD, Src0 * Src1), reference=_ref)
    row = dve_ops._CUSTOM_DVE_ROW_BASE + len(dve_ops.OPS)
    shas = {}
    for ver in ("v3", "v4"):
        try:
            uops = lower(spec, ver=ver)
            shas[ver] = DveOpSpec(name=NAME, opcode=row, uops=uops,
                                  rd1_en=True).sha(ver)
        except Exception:
            pass
    op = dve_ops.DveOp(NAME, spec, subdim=False, uops_sha=shas)
    dve_ops.OPS.append(op)
    dve_ops.CUSTOM_DVE_SPECS[NAME] = spec
    dve_ops._SUB_OPCODE_FOR_NAME[NAME] = row
    return op


# ------------------------------------------------------------- device program
def build_nc(meta, no_collective=False):
    import concourse.bass as bass  # noqa: F401
    import concourse.tile as tile
    from concourse import mybir, bacc
    from concourse.masks import make_identity

    f32 = mybir.dt.float32
    ALU = mybir.AluOpType
    AX = mybir.AxisListType
    AF = mybir.ActivationFunctionType

    n_c, E_pad, T = meta["n_c"], meta["E_pad"], meta["T"]
    wb, n_chunks = meta["wb"], meta["n_chunks"]
    first_t, last_t = meta["first_t"], meta["last_t"]
    win, chk, N, n_cores = meta["win"], meta["chk"], meta["N"], meta["n_cores"]

    nc = bacc.Bacc("TRN2", target_bir_lowering=False, debug=False,
                   num_devices=n_cores)

    ea_d = nc.dram_tensor("ea", [E_pad, 128], f32, kind="ExternalInput")
    xg_d = nc.dram_tensor("xg", [E_pad, 128], f32, kind="ExternalInput")
    shls_d = nc.dram_tensor("shls", [E_pad, 8], f32, kind="ExternalInput")
    w1_d = nc.dram_tensor("w1", [128, 128], f32, kind="ExternalInput")
    b1_d = nc.dram_tensor("b1", [128, 1], f32, kind="ExternalInput")
    w2_d = nc.dram_tensor("w2", [128, 4096], f32, kind="ExternalInput")
    b2p_d = nc.dram_tensor("b2p", [128, 256], f32, kind="ExternalInput")
    iota_d = nc.dram_tensor("iota", [P, win], f32, kind="ExternalInput")
    cnst_d = nc.dram_tensor("cnst", [1, 128], f32, kind="ExternalInput")
    invc_d = nc.dram_tensor("invc", [n_c, 1], f32, kind="ExternalInput")
    resid_d = nc.dram_tensor("resid", [n_c, 128], f32, kind="ExternalInput")
    out_d = nc.dram_tensor("out", [n_c, 128], f32, kind="ExternalOutput")

    n_node_tiles = (n_c + P - 1) // P

    with tile.TileContext(nc, num_cores=n_cores) as tc:
        with (
            tc.tile_pool(name="const", bufs=1) as cst,
            tc.tile_pool(name="io", bufs=3) as io,
            tc.tile_pool(name="sb", bufs=2) as sb,
            tc.tile_pool(name="xbp", bufs=n_node_tiles) as xbp,
            tc.tile_pool(name="pss", bufs=2, space="PSUM") as pss,
            tc.tile_pool(name="psw", bufs=2, space="PSUM") as psw,
            tc.tile_pool(name="pscat", bufs=2, space="PSUM") as pscat,
            tc.tile_pool(name="dram", bufs=1, space="DRAM") as dram,
        ):
            # ---- constants
            w1_sb = cst.tile([128, 128], f32, tag="w1")
            nc.sync.dma_start(out=w1_sb[:], in_=w1_d[:])
            b1_sb = cst.tile([128, 1], f32, tag="b1")
            nc.sync.dma_start(out=b1_sb[:], in_=b1_d[:])
            w2_sb = cst.tile([128, 4096], f32, tag="w2")
            nc.sync.dma_start(out=w2_sb[:], in_=w2_d[:])
            b2p_sb = cst.tile([128, 256], f32, tag="b2p")
            nc.sync.dma_start(out=b2p_sb[:], in_=b2p_d[:])
            iota_sb = cst.tile([P, win], f32, tag="iota")
            nc.sync.dma_start(out=iota_sb[:], in_=iota_d[:])
            cnst_sb = cst.tile([1, 128], f32, tag="cnst")
            nc.sync.dma_start(out=cnst_sb[:], in_=cnst_d[:])
            ident = cst.tile([128, 128], f32, tag="ident")
            make_identity(nc, ident[:])
            zeros_sb = cst.tile([128, chk], f32, tag="zeros")
            nc.gpsimd.memset(zeros_sb[:], 0.0)
            ones_sb = cst.tile([128, 1], f32, tag="ones")
            nc.gpsimd.memset(ones_sb[:], 1.0)
            stats_acc = cst.tile([96, 1], f32, tag="stacc")
            nc.gpsimd.memset(stats_acc[:], 0.0)

            lbuf = cst.tile([128, 6 * 33], f32, tag="lbuf")
            nc.gpsimd.memset(lbuf[:], 0.0)

            chunk_tiles = [None] * n_chunks
            xb_tiles = []
            xb_rows = []

            def finalize_chunk(c):
                nvalid = min(chk, n_c - c * chk)
                cs = sb.tile([128, chk], f32, tag="chfin")
                nc.scalar.copy(cs[:, 0:nvalid], chunk_tiles[c][:, 0:nvalid])
                nsub = (nvalid + P - 1) // P
                for j in range(nsub):
                    rows = min(P, nvalid - j * P)
                    node0 = c * chk + j * P
                    ntp = pss.tile([128, 128], f32, tag="pss")
                    nc.tensor.transpose(
                        out=ntp[0:rows, :], in_=cs[:, j * P:j * P + rows],
                        identity=ident[:])
                    invc_t = io.tile([128, 1], f32, tag="invc")
                    nc.sync.dma_start(out=invc_t[0:rows, :],
                                      in_=invc_d[node0:node0 + rows, :])
                    resid_t = io.tile([128, 128], f32, tag="resid")
                    nc.sync.dma_start(out=resid_t[0:rows, :],
                                      in_=resid_d[node0:node0 + rows, :])
                    xb = xbp.tile([128, 128], f32, tag="xb")
                    nc.vector.scalar_tensor_tensor(
                        out=xb[0:rows, :], in0=ntp[0:rows, :],
                        scalar=invc_t[0:rows, 0:1], in1=resid_t[0:rows, :],
                        op0=ALU.mult, op1=ALU.add)
                    xb_tiles.append(xb)
                    xb_rows.append((node0, rows))
                    # stats block [rows, 96] = [s | s^2 | sum_i v^2]
                    stt = sb.tile([128, 96], f32, tag="stt")
                    nc.scalar.copy(stt[0:rows, 0:32], xb[0:rows, 0:32])
                    nc.scalar.square(stt[0:rows, 32:64], xb[0:rows, 0:32])
                    v2 = sb.tile([128, 96], f32, tag="v2")
                    nc.scalar.square(v2[0:rows, :], xb[0:rows, 32:128])
                    nc.vector.tensor_reduce(
                        out=stt[0:rows, 64:96],
                        in_=v2[0:rows, :].rearrange("e (u i) -> e u i", u=32, i=3),
                        axis=AX.X, op=ALU.add)
                    stp = pss.tile([96, 1], f32, tag="pss")
                    nc.tensor.matmul(
                        out=stp[:], lhsT=stt[0:rows, 0:96],
                        rhs=ones_sb[0:rows, 0:1], start=True, stop=True)
                    nc.vector.tensor_tensor(
                        out=stats_acc[:], in0=stats_acc[:], in1=stp[:],
                        op=ALU.add)

            # ---------------- main edge-tile loop
            for t in range(T):
                ea_t = io.tile([128, 128], f32, tag="ea")
                nc.sync.dma_start(out=ea_t[:], in_=ea_d[t * P:(t + 1) * P, :])
                xg_t = io.tile([128, 128], f32, tag="xg")
                nc.sync.dma_start(out=xg_t[:], in_=xg_d[t * P:(t + 1) * P, :])
                shls_t = io.tile([128, 8], f32, tag="shls")
                nc.sync.dma_start(out=shls_t[:], in_=shls_d[t * P:(t + 1) * P, :])

                # PE: eaT -> mm1 -> relu
                eaT_ps = pss.tile([128, 128], f32, tag="pss")
                nc.tensor.transpose(out=eaT_ps[:], in_=ea_t[:], identity=ident[:])
                eaT_sb = sb.tile([128, 128], f32, tag="eaT")
                nc.scalar.copy(eaT_sb[:], eaT_ps[:])
                hT_ps = pss.tile([128, 128], f32, tag="pss")
                nc.tensor.matmul(out=hT_ps[:], lhsT=w1_sb[:], rhs=eaT_sb[:],
                                 start=True, stop=True)
                hT_sb = sb.tile([128, 128], f32, tag="hT")
                nc.scalar.activation(hT_sb[:], hT_ps[:], AF.Relu,
                                     bias=b1_sb[:, 0:1])

                # DVE: V prep
                V = sb.tile([128, 256], f32, tag="V")
                x0 = xg_t[:, 0:32]
                x1v = xg_t[:, 32:128].rearrange("e (u i) -> e u i", u=32, i=3)
                sh0 = shls_t[:, 0:1]
                sh1u = shls_t[:, 1:4].unsqueeze(1).broadcast_to([P, 32, 3])
                y2t = sb.tile([128, 96], f32, tag="y2t")
                y2tv = y2t[:].rearrange("e (u i) -> e u i", u=32, i=3)
                nc.vector.tensor_tensor(out=y2tv, in0=x1v, in1=sh1u, op=ALU.mult)
                nc.vector.tensor_reduce(out=V[:, 32:64], in_=y2tv,
                                        axis=AX.X, op=ALU.add)
                nc.scalar.mul(V[:, 0:32], x0, sh0)
                x0u = x0.unsqueeze(2).broadcast_to([P, 32, 3])
                nc.vector.tensor_tensor(
                    out=V[:, 64:160].rearrange("e (u i) -> e u i", u=32, i=3),
                    in0=x0u, in1=sh1u, op=ALU.mult)
                nc.scalar.mul(V[:, 160:256], xg_t[:, 32:128], sh0)

                # PE: transpose V, corr matmuls
                VT_ps = pss.tile([128, 256], f32, tag="pss")
                nc.tensor.transpose(out=VT_ps[:, 0:128], in_=V[:, 0:128],
                                    identity=ident[:])
                nc.tensor.transpose(out=VT_ps[:, 128:256], in_=V[:, 128:256],
                                    identity=ident[:])
                VT_sb = sb.tile([128, 256], f32, tag="VT")
                nc.scalar.copy(VT_sb[:], VT_ps[:])
                corr_ps = pss.tile([128, 128], f32, tag="pss")
                nc.tensor.matmul(out=corr_ps[:], lhsT=VT_sb[:, 0:128],
                                 rhs=b2p_sb[:, 0:128], start=True, stop=False)
                nc.tensor.matmul(out=corr_ps[:], lhsT=VT_sb[:, 128:256],
                                 rhs=b2p_sb[:, 128:256], start=False, stop=True)

                # mm2 + fused TP contraction per path:
                # cumsum(in0*in1) over (w' outer, u inner) stream, then
                # sample every 32nd running value and difference.
                cop = _register_mul_cumsum()
                red = {}

                def cumsum_reduce(wps, in1v, slot, csbuf, csoff, rout):
                    csv = csbuf[:, csoff:csoff + 1024].rearrange(
                        "e (w u) -> e w u", w=32, u=32)
                    wv = wps[:].rearrange("e (u w) -> e w u", u=32, w=32)
                    nc.vector._custom_dve(cop, out=csv, in0=wv, in1=in1v)
                    base = slot * 33
                    nc.scalar.copy(
                        lbuf[:, base + 1:base + 33].unsqueeze(2),
                        csv[:, :, 31:32])
                    nc.vector.tensor_tensor(
                        out=rout, in0=lbuf[:, base + 1:base + 33],
                        in1=lbuf[:, base:base + 32], op=ALU.subtract)

                for p in range(4):
                    wps = psw.tile([128, 1024], f32, tag="w")
                    for h in range(2):
                        nc.tensor.matmul(
                            out=wps[:, h * 512:(h + 1) * 512], lhsT=hT_sb[:],
                            rhs=w2_sb[:, p * 1024 + h * 512:p * 1024 + (h + 1) * 512],
                            start=True, stop=True)
                    cs = sb.tile([128, 3072], f32, tag="prod")
                    if p == 2:  # path C: one call per vector component i
                        rC = sb.tile([128, 96], f32, tag="rC")  # (i, w') layout
                        vCiv = V[:, 160:256].rearrange(
                            "e (u i) -> e i u", u=32, i=3)
                        for i in range(3):
                            cumsum_reduce(
                                wps,
                                vCiv[:, i:i + 1, :].broadcast_to([P, 32, 32]),
                                3 + i, cs, i * 1024,
                                rC[:, i * 32:(i + 1) * 32])
                        red[p] = rC
                    else:
                        if p == 0:
                            vec = V[:, 0:32]
                        elif p == 1:
                            vec = xg_t[:, 0:32]
                        else:
                            vec = V[:, 32:64]
                        r = sb.tile([128, 32], f32, tag=f"r{p}")
                        cumsum_reduce(
                            wps, vec.unsqueeze(1).broadcast_to([P, 32, 32]),
                            p, cs, 0, r[:])
                        red[p] = r

                # assembly
                scat_sb = sb.tile([128, 128], f32, tag="scat")
                o0 = sb.tile([128, 32], f32, tag="o0")
                nc.vector.tensor_tensor(out=o0[:], in0=red[0][:], in1=red[3][:],
                                        op=ALU.add)
                nc.vector.tensor_tensor(out=scat_sb[:, 0:32], in0=o0[:],
                                        in1=corr_ps[:, 0:32], op=ALU.add)
                t1 = sb.tile([128, 96], f32, tag="t1")
                cBv = red[1][:].unsqueeze(2).broadcast_to([P, 32, 3])
                nc.vector.tensor_tensor(
                    out=t1[:].rearrange("e (w i) -> e w i", w=32, i=3),
                    in0=cBv, in1=sh1u, op=ALU.mult)
                t2 = sb.tile([128, 96], f32, tag="t2")
                nc.vector.tensor_tensor(
                    out=t2[:].rearrange("e (w i) -> e w i", w=32, i=3),
                    in0=t1[:].rearrange("e (w i) -> e w i", w=32, i=3),
                    in1=red[2][:].rearrange("e (i w) -> e w i", i=3, w=32),
                    op=ALU.add)
                nc.vector.tensor_tensor(out=scat_sb[:, 32:128], in0=t2[:],
                                        in1=corr_ps[:, 32:128], op=ALU.add)

                # one-hot S and scatter matmuls
                S_sb = sb.tile([P, win], f32, tag="S")
                nc.vector.tensor_scalar(out=S_sb[:], in0=iota_sb[:],
                                        scalar1=shls_t[:, 4:5], scalar2=None,
                                        op0=ALU.is_equal)
                for c in range(n_chunks):
                    lo, hi = c * chk, min((c + 1) * chk, n_c)
                    a, b = max(wb[t], lo), min(wb[t] + win, hi)
                    if a >= b:
                        continue
                    if t == first_t[c]:
                        chunk_tiles[c] = pscat.tile([128, chk], f32, tag="ch", name=f"ch{c}")
                        nc.tensor.matmul(out=chunk_tiles[c][:],
                                         lhsT=ident[:], rhs=zeros_sb[:],
                                         start=True, stop=False)
                    nc.tensor.matmul(
                        out=chunk_tiles[c][:, a - lo:b - lo],
                        lhsT=scat_sb[:], rhs=S_sb[:, a - wb[t]:b - wb[t]],
                        start=False, stop=(t == last_t[c]))
                for c in range(n_chunks):
                    if last_t[c] == t:
                        finalize_chunk(c)

            # ---------------- tail: AllReduce of stats, normalize, write out
            arin = dram.tile([96, 1], f32, name="arin")
            arout = dram.tile([96, 1], f32, name="arout")
            nc.sync.dma_start(out=arin[:], in_=stats_acc[:])
            if no_collective:
                nc.sync.dma_start(out=arout[:], in_=arin[:])
            else:
                from concourse import mybir as _mb
                nc.gpsimd.collective_compute(
                    "AllReduce", _mb.AluOpType.add,
                    replica_groups=[list(range(n_cores))],
                    ins=[arin[:].opt()], outs=[arout[:].opt()])
            srow = sb.tile([1, 96], f32, tag="srow")
            nc.sync.dma_start(out=srow[:], in_=arout[:].rearrange("a b -> b a"))

            # constants prep on partition 0
            pr = sb.tile([1, 160], f32, tag="pr")
            mu = pr[:, 0:32]
            alpha = pr[:, 32:64]
            gamma = pr[:, 64:96]
            delta = pr[:, 96:128]
            tmp = pr[:, 128:160]
            nc.vector.tensor_scalar(out=mu, in0=srow[:, 0:32], scalar1=1.0 / N,
                                    scalar2=None, op0=ALU.mult)
            # var = S2/N - mu^2 + eps
            nc.vector.tensor_scalar(out=tmp, in0=srow[:, 32:64], scalar1=1.0 / N,
                                    scalar2=EPS, op0=ALU.mult, op1=ALU.add)
            va = sb.tile([1, 32], f32, tag="va")
            nc.vector.tensor_tensor(out=va[:], in0=mu, in1=mu, op=ALU.mult)
            nc.vector.tensor_tensor(out=tmp, in0=tmp, in1=va[:], op=ALU.subtract)
            nc.scalar.sqrt(tmp, tmp)
            nc.vector.reciprocal(tmp, tmp)
            nc.vector.tensor_tensor(out=alpha, in0=tmp, in1=cnst_sb[:, 0:32],
                                    op=ALU.mult)
            nc.vector.tensor_scalar(out=tmp, in0=srow[:, 64:96],
                                    scalar1=1.0 / (3 * N), scalar2=EPS,
                                    op0=ALU.mult, op1=ALU.add)
            nc.scalar.sqrt(tmp, tmp)
            nc.vector.reciprocal(tmp, tmp)
            nc.vector.tensor_tensor(out=gamma, in0=tmp, in1=cnst_sb[:, 32:64],
                                    op=ALU.mult)
            nc.vector.tensor_tensor(out=delta, in0=mu, in1=alpha, op=ALU.mult)
            nc.vector.tensor_tensor(out=delta, in0=delta, in1=cnst_sb[:, 64:96],
                                    op=ALU.subtract)

            rows2 = sb.tile([1, 256], f32, tag="rows2")
            nc.gpsimd.memset(rows2[:], 0.0)
            nc.vector.tensor_copy(rows2[:, 0:32], alpha)
            nc.vector.tensor_copy(
                rows2[:, 32:128].rearrange("e (u i) -> e u i", u=32, i=3),
                gamma.unsqueeze(2).broadcast_to([1, 32, 3]))
            nc.vector.tensor_copy(rows2[:, 128:160], delta)
            rowb = dram.tile([1, 256], f32, name="rowb")
            nc.sync.dma_start(out=rowb[:], in_=rows2[:])
            scaleB = cst.tile([128, 128], f32, tag="scaleB")
            nc.sync.dma_start(
                out=scaleB[:].unsqueeze(1),
                in_=rowb[0:1, 0:128].partition_broadcast(128))
            deltaB = cst.tile([128, 128], f32, tag="deltaB")
            nc.sync.dma_start(
                out=deltaB[:].unsqueeze(1),
                in_=rowb[0:1, 128:256].partition_broadcast(128))

            for xb, (node0, rows) in zip(xb_tiles, xb_rows):
                nrm = sb.tile([128, 128], f32, tag="nrm")
                nc.vector.tensor_tensor(out=nrm[0:rows, :], in0=xb[0:rows, :],
                                        in1=scaleB[0:rows, :], op=ALU.mult)
                nrm2 = sb.tile([128, 128], f32, tag="nrm2")
                nc.vector.tensor_tensor(out=nrm2[0:rows, :], in0=nrm[0:rows, :],
                                        in1=deltaB[0:rows, :], op=ALU.subtract)
                nc.sync.dma_start(out=out_d[node0:node0 + rows, :],
                                  in_=nrm2[0:rows, :])

    nc.compile()
    return nc


# ------------------------------------------------------------------ entry
_TRACE = False
_LAST = {}


def kernel(**inputs):
    from concourse.bass_utils import run_bass_kernel_spmd

    cores, consts, meta = host_prep(inputs)
    key = (meta["E_pad"], meta["wb"], meta["first_t"], meta["last_t"],
           meta["n_c"], meta["N"])
    if key not in _CACHE:
        _CACHE[key] = build_nc(meta)
    nc = _CACHE[key]

    in_maps = []
    for ci in range(meta["n_cores"]):
        m = {"ea": cores[ci]["ea"], "xg": cores[ci]["xg"],
             "shls": cores[ci]["shls"], "invc": cores[ci]["invc"],
             "resid": cores[ci]["resid"], "w1": consts["w1"],
             "b1": consts["b1"], "w2": consts["w2"], "b2p": consts["b2p"],
             "iota": consts["iota"], "cnst": consts["cnst"]}
        in_maps.append(m)
    res = run_bass_kernel_spmd(nc, in_maps,
                               core_ids=list(range(meta["n_cores"])),
                               trace=_TRACE)
    _LAST["exec_time_ns"] = res.exec_time_ns
    _LAST["profile_json"] = res.profile_json
    out = np.concatenate([res.results[ci]["out"]
                          for ci in range(meta["n_cores"])], axis=0)
    return out.astype(np.float32)
